# revision 13
# baseline (speedup 1.0000x reference)
"""BCOP forward on 8 TRN2 cores — factored-projector formulation.

Math (validated vs reference in fp64 numpy, rel 4e-8):
  - The BCOP kernel is p3 = b1 (x) b2 (matrix convolution) with
    b[i,j] = A_i B_j, A_0 = P, A_1 = I-P (projections, B_0+B_1 = I), and
    the conv weight W[co,ci,kh,kw] = (H @ p3[kw,kh])[ci,co].  The conv
    therefore factorizes into 5 chained channel-matrix stages applied to
    the image:
      u0 = H^T x + bias          (bias commutes through all later stages)
      u1 = S1c u0 + P1 (u0 - S1c u0)     (col shifts {0,+1})
      u2 = S1r u1 + P2 (u1 - S1r u1)     (row shifts {0,+1})
      u3 = u2 + P3 (S-1c u2 - u2)        (col shifts {-1,0})
      out = u3 + P4 (S-1r u3 - u3)       (row shifts {-1,0})
    where P_i = Z_i Z_i^T, Z_i = ortho[i][:, :128], all shifts circular.
    5 CxC matmul units per pixel instead of 9.  P_i is applied DENSE (the
    PE is row-bound, so rank-128 factoring saves nothing and would cost an
    extra PSUM evacuation).
  - The "S u +" term of stages 3/4 is accumulated in PSUM by an identity
    matmul (PE has slack); stages 1/2 use a DVE tensor_add on evacuation.
    Engine balance per image: PE ~41us, DVE ~29us, ACT ~26us, Pool ~35us.
  - sigma via repeated squaring; 20 Bjorck iters in fp32r (must be exact:
    truncating to 18 iters gives 2.6e-2 > the 2e-2 gate).
  - Stages run in bf16 (7.6e-3 total rel err vs the 2e-2 gate).

Parallelization: weight construction sharded across cores by matrix
(core i builds matrix i mod 5); an 8-rank AllGather shares the WT halves
(WT_k[:128,:] k=0..4 from cores 0-4, WT_0[128:,:] from core 5); each
core then assembles P_i = WTh_i^T WTh_i and H locally.  Conv is
data-parallel over batch (4 images per core).
"""

import numpy as np

import concourse.bass as bass
import concourse.mybir as mybir
import concourse.tile as tile
from concourse import bacc
from concourse.bass_utils import run_bass_kernel_spmd

P = 128
C = 256
NK = 5
N_CORES = 8
B_TOTAL = 32
B_CORE = B_TOTAL // N_CORES
H = 64
NPIX = H * H
BJORCK_ITERS = 20

F32 = mybir.dt.float32
F32R = mybir.dt.float32r
BF16 = mybir.dt.bfloat16
ALU = mybir.AluOpType
ACTF = mybir.ActivationFunctionType


def build_body(tc, out_ap, xs, pmk, u0k, bias_ap, ctx):
    nc = tc.nc
    from concourse.masks import make_identity

    persist = ctx.enter_context(tc.tile_pool(name="persist", bufs=1))
    small = ctx.enter_context(tc.tile_pool(name="small", bufs=3))

    U_sb = persist.tile([P, 2, 1], F32R)
    bias_sb = persist.tile([P, 2, 1], F32)

    # ---- constants ----
    ID1 = persist.tile([P, P], F32)
    make_identity(nc, ID1)
    IDb = persist.tile([P, P], BF16)
    nc.vector.tensor_copy(IDb[:], ID1[:])
    I15 = persist.tile([P, 2, C], F32)
    nc.vector.memset(I15[:], 0.0)
    for mt in range(2):
        nc.scalar.mul(I15[:, mt, mt * P:(mt + 1) * P], ID1[:], 1.5)
    I15f = I15.rearrange("p a b -> p (a b)")

    RB = persist.tile([P, 1], F32)            # broadcast 1/s (own matrix)
    # stage weights (bf16)
    W0_sb = persist.tile([P, 2, C], F32R)     # H rows: W0_sb[p,tr,f]=H[p+tr*128,f]
    Pd_sb = persist.tile([P, 4, 2, C], BF16)  # P_i rows: [p,i,tr,c]=P_i[tr*128+p,c]
    WTag = persist.tile([P, 6, C], F32R)      # AG: 0..4 = WT_k[:128,:], 5 = WT_0[128:,:]

    xpool = ctx.enter_context(tc.tile_pool(name="xpool", bufs=2))
    xf = []

    def flat(ap3):
        return ap3.rearrange("p a b -> p (a b)")

    def prod_mms(out_ps, X3, Y3, n_tr=2):
        """dst[mt] += X[tr][:, mt]^T @ Y[tr]; ONE accumulation group per bank."""
        first = True
        for mt in range(2):
            for tr in range(n_tr):
                last = (mt == 1 and tr == n_tr - 1)
                nc.tensor.matmul(out_ps[:, mt * C:(mt + 1) * C],
                                 X3[:, tr, mt * P:(mt + 1) * P], Y3[:, tr, :],
                                 start=first, stop=last)
                first = False

    with tc.tile_pool(name="build", bufs=1) as build, \
         tc.tile_pool(name="wstate", bufs=2) as wpool, \
         tc.tile_pool(name="mpool", bufs=4) as mpool, \
         tc.tile_pool(name="vpool", bufs=8) as vpool, \
         tc.tile_pool(name="ccdram", bufs=1, space="DRAM") as ccdram:

        Wcur = wpool.tile([P, 2, C], F32R, tag="W")
        WTcur = wpool.tile([P, 2, C], F32R, tag="WT")
        G_sb = build.tile([P, 2, C], F32R)
        G2_sb = build.tile([P, 2, C], F32R)
        G16_sb = build.tile([P, 2, C], F32R)

        for tr in range(2):
            nc.sync.dma_start(Wcur[:, tr, :], pmk[tr * P:(tr + 1) * P, :])
        for tr in range(2):
            nc.sync.dma_start(U_sb[:, tr, :], u0k[tr * P:(tr + 1) * P, :])
        for mt in range(2):
            nc.sync.dma_start(bias_sb[:, mt, :],
                              bias_ap[mt * P:(mt + 1) * P].unsqueeze(1))

        # ============ phase 1: sigma via repeated squaring (own matrix) ======
        with tc.tile_pool(name="ps1", bufs=2, space="PSUM") as ps1:
            gps = ps1.tile([P, 2 * C], F32, tag="sq")
            prod_mms(gps, Wcur, Wcur)
            nc.scalar.copy(flat(G_sb), gps[:])

            prev = G_sb
            for pw in (2, 4, 8, 16):
                sq = ps1.tile([P, 2 * C], F32, tag="sq", name=f"sq{pw}")
                prod_mms(sq, prev, prev)
                if pw == 2:
                    dst = G2_sb
                elif pw == 16:
                    dst = G16_sb
                else:
                    dst = build.tile([P, 2, C], F32R, tag="gtmp",
                                     name=f"g{pw}", bufs=2)
                if pw in (4, 16):
                    nc.vector.tensor_copy(flat(dst), sq[:])
                else:
                    nc.scalar.copy(flat(dst), sq[:])
                prev = dst

            def matvec(G3, vin, nm):
                vout = vpool.tile([P, 2], F32R, tag="v", name=f"v_{nm}")
                for mt in range(2):
                    vps = ps1.tile([P, 1], F32, tag="vps", bufs=4,
                                   name=f"vp_{nm}_{mt}")
                    for tr in range(2):
                        nc.tensor.matmul(
                            vps[:], G3[:, tr, mt * P:(mt + 1) * P].bitcast(F32),
                            vin[:, tr:tr + 1].bitcast(F32),
                            start=(tr == 0), stop=(tr == 1))
                    nc.scalar.copy(vout[:, mt:mt + 1], vps[:])
                return vout

            v1 = matvec(Wcur, U_sb, "v1")
            m1 = matvec(G2_sb, v1, "m1")
            m2 = matvec(G16_sb, m1, "m2")
            m3 = matvec(G_sb, m2, "m3")

            def dot(va, vb, nm):
                dps = ps1.tile([1, 1], F32, tag="vps", bufs=4, name=f"d_{nm}")
                for tr in range(2):
                    nc.tensor.matmul(dps[:], va[:, tr:tr + 1].bitcast(F32),
                                     vb[:, tr:tr + 1].bitcast(F32),
                                     start=(tr == 0), stop=(tr == 1))
                return dps

            dps0 = dot(v1, m2, "0")
            dps1 = dot(v1, m3, "1")
            dsb = small.tile([1, 3], F32, tag="dsb")
            nc.vector.tensor_copy(dsb[:, 0:1], dps0[:])
            nc.vector.reciprocal(dsb[:, 1:2], dps1[:])
            nc.vector.tensor_mul(dsb[:, 2:3], dsb[:, 0:1], dsb[:, 1:2])
            rsb = small.tile([1, 1], F32, tag="rsb")
            nc.scalar.sqrt(rsb[:], dsb[:, 2:3])
            nc.gpsimd.partition_broadcast(RB[:, 0:1], rsb[:])
            # W0 = A * r (in place), then WT0 = W0^T via PE transpose
            nc.vector.tensor_scalar_mul(Wcur[:], Wcur[:], RB[:, 0:1])
            for tr in range(2):
                for mt in range(2):
                    tps = ps1.tile([P, P], F32, tag="tp")
                    nc.tensor.transpose(
                        tps[:], Wcur[:, tr, mt * P:(mt + 1) * P].bitcast(F32),
                        ID1[:])
                    nc.scalar.copy(WTcur[:, mt, tr * P:(tr + 1) * P], tps[:])

        # x pair 1: ACT-triggered DMAs — the ACT queue reaches these at
        # sigma end, so the packets flow during Bjorck (PE-only phase).
        for b in range(2):
            xft = xpool.tile([P, 2, NPIX], F32R, tag="x", name=f"x_{b}")
            for tr in range(2):
                nc.scalar.dma_start(
                    xft[:, tr, :].rearrange("p (h w) -> p h w", w=H),
                    xs[b, tr * P:(tr + 1) * P, :, :])
            xf.append(xft)

        # ================= phase 2: Bjorck (own matrix) =================
        with tc.tile_pool(name="ps2", bufs=2, space="PSUM") as ps2:
            for it in range(BJORCK_ITERS):
                last = it == BJORCK_ITERS - 1
                Wnxt = None if last else wpool.tile([P, 2, C], F32R, tag="W",
                                                    name=f"W_{it}")
                WTnxt = wpool.tile([P, 2, C], F32R, tag="WT", name=f"WT_{it}")
                gps = ps2.tile([P, 2 * C], F32, tag="g", bufs=2)
                prod_mms(gps, Wcur, Wcur)
                m_sb = mpool.tile([P, 2 * C], F32R, tag="m", name=f"m_{it}")
                nc.vector.scalar_tensor_tensor(
                    m_sb[:, 0:C], gps[:, 0:C], -0.5, I15f[:, 0:C],
                    op0=ALU.mult, op1=ALU.add)
                nc.vector.scalar_tensor_tensor(
                    m_sb[:, C:2 * C], gps[:, C:2 * C], -0.5, I15f[:, C:2 * C],
                    op0=ALU.mult, op1=ALU.add)
                m3 = m_sb.rearrange("p (a b) -> p a b", b=C)
                if not last:
                    wps = ps2.tile([P, 2 * C], F32, tag="w", bufs=2)
                    prod_mms(wps, WTcur, m3)
                    nc.scalar.copy(flat(Wnxt), wps[:])
                wtps = ps2.tile([P, 2 * C], F32, tag="wt", bufs=2)
                prod_mms(wtps, m3, WTcur)
                nc.vector.tensor_copy(flat(WTnxt), wtps[:])
                if Wnxt is not None:
                    Wcur = Wnxt
                WTcur = WTnxt

        # ============ AllGather the needed WT halves across cores ============
        cc_in = ccdram.tile([1, P * C], F32R)
        cc_out = ccdram.tile([N_CORES, P * C], F32R, addr_space="Shared")
        pid = nc.sync.partition_id()
        nc.sync.dma_start(cc_in[0].rearrange("(p n) -> p n", p=P),
                          WTcur[:, 0, :], cond=(pid != 5))
        nc.sync.dma_start(cc_in[0].rearrange("(p n) -> p n", p=P),
                          WTcur[:, 1, :], cond=(pid == 5))
        # x pair 2: fenced behind cc_in on the sync queue -> flows during
        # the AllGather window.
        for b in range(2, B_CORE):
            xft = xpool.tile([P, 2, NPIX], F32R, tag="x", name=f"x_{b}")
            for tr in range(2):
                nc.sync.dma_start(
                    xft[:, tr, :].rearrange("p (h w) -> p h w", w=H),
                    xs[b, tr * P:(tr + 1) * P, :, :])
            xf.append(xft)
        nc.gpsimd.collective_compute(
            "AllGather", ALU.bypass, ins=[cc_in.opt()], outs=[cc_out.opt()],
            replica_groups=[list(range(N_CORES))])
        for k in range(6):
            nc.sync.dma_start(WTag[:, k, :],
                              cc_out[k].rearrange("(p n) -> p n", p=P))

        # ============ stage-weight prep ============
        # P_i = WTh_i^T @ WTh_i (dense, symmetric); H rows via PE transpose.
        with tc.tile_pool(name="ps3", bufs=4, space="PSUM") as ps3:
            for i in range(4):
                for mt in range(2):
                    pps = ps3.tile([P, C], F32, tag="pp", name=f"p_{i}_{mt}")
                    nc.tensor.matmul(pps[:],
                                     WTag[:, i + 1, mt * P:(mt + 1) * P],
                                     WTag[:, i + 1, :], start=True, stop=True)
                    nc.scalar.copy(Pd_sb[:, i, mt, :], pps[:])
            # W0_sb[:,tr,c*128:...] = (T0{a,b=c}[:, tr*128:...])^T
            for tr in range(2):
                for cchunk in range(2):
                    src = WTag[:, 0 if cchunk == 0 else 5,
                               tr * P:(tr + 1) * P].bitcast(F32)
                    tps = ps3.tile([P, P], F32, tag="tp", name=f"w0_{tr}_{cchunk}")
                    nc.tensor.transpose(tps[:], src, ID1[:])
                    nc.scalar.copy(W0_sb[:, tr, cchunk * P:(cchunk + 1) * P],
                                   tps[:])

    # ================= stages =================
    # u tiles: uniform [P, 2, 65, 65] bf16.  Storage conventions:
    #   u0: cols 0..63 data, col 64 wrap;   u1: rows 0..63 data, row 64 wrap
    #   u2: cols 1..64 data, col 0 wrap;    u3: rows 1..64 data, row 0 wrap
    # With these, every stage reads  d = t[.., a0:a0+8, 0:64]-ish windows with
    # identical slice patterns (see stageP).
    upool = ctx.enter_context(tc.tile_pool(name="upool", bufs=4))
    dpool = ctx.enter_context(tc.tile_pool(name="dpool", bufs=2))
    opool = ctx.enter_context(tc.tile_pool(name="opool", bufs=3))

    with tc.tile_pool(name="psO", bufs=3, space="PSUM") as psO:

        def stage0(b):
            """u0 = H^T x + bias; writes rows r, cols 0..63; wrap col 64."""
            u0 = upool.tile([P, 2, 65, 65], BF16, tag="u", name=f"u0_{b}")
            for blk in range(8):
                r0 = blk * 8
                ops = psO.tile([P, 1024], F32, tag="o", name=f"s0_{b}_{blk}")
                for mt in range(2):
                    for tr in range(2):
                        nc.tensor.matmul(
                            ops[:, mt * 512:(mt + 1) * 512],
                            W0_sb[:, tr, mt * P:(mt + 1) * P],
                            xf[b][:, tr, r0 * H:(r0 + 8) * H],
                            start=(tr == 0), stop=(tr == 1))
                src = ops[:].rearrange("p (m h w) -> p m h w", m=2, w=H)
                for mt in range(2):
                    nc.scalar.activation(u0[:, mt, r0:r0 + 8, 0:64], src[:, mt],
                                         ACTF.Identity, bias=bias_sb[:, mt, :],
                                         scale=1.0)
            for mt in range(2):
                for hh in range(2):
                    r = slice(hh * 32, hh * 32 + 32)
                    nc.scalar.dma_start(u0[:, mt, r, 64:65], u0[:, mt, r, 0:1])
            return u0

        def emit_diff(uS, kind, b, s):
            """Half-image diffs for stage s, emitted right after image b's
            previous stage so they don't queue behind the other image's
            DVE evacuations."""
            d = dpool.tile([P, 2, 64, 64], BF16, tag="d", name=f"d{s}_{b}")
            for hh in range(2):
                r = slice(hh * 32, hh * 32 + 32)
                r1 = slice(hh * 32 + 1, hh * 32 + 33)
                if kind == "col":
                    nc.vector.tensor_sub(d[:, :, r, :], uS[:, :, r, 0:64],
                                         uS[:, :, r, 1:65])
                else:
                    nc.vector.tensor_sub(d[:, :, r, :], uS[:, :, r, 0:64],
                                         uS[:, :, r1, 0:64])
            return d

        def stageP(uS, d, i, kind, b, s, final=False):
            """out = add + P_i d.  Slice patterns are uniform in the tile
            frame: d = uS[.., w0:w0+?, 0:64]-style, add = shifted window.

            s in (1, 2): evacuate with DVE tensor_add (psum + add).
            s in (3, 4): identity matmul accumulates `add` into PSUM; plain
            ACT copy evacuates (s==4 to fp32 osb -> DMA out).
            """
            uN = None
            if not final:
                uN = upool.tile([P, 2, 65, 65], BF16, tag="u", name=f"u{s}_{b}")
            df = d.rearrange("p a h w -> p a (h w)")
            for blk in range(8):
                r0 = blk * 8
                ops = psO.tile([P, 1024], F32, tag="o", name=f"o{s}_{b}_{blk}")
                if kind == "col":
                    add = uS[:, :, r0:r0 + 8, 1:65]
                else:
                    add = uS[:, :, r0 + 1:r0 + 9, 0:64]
                for mt in range(2):
                    nc.tensor.matmul(ops[:, mt * 512:(mt + 1) * 512],
                                     Pd_sb[:, i, 0, mt * P:(mt + 1) * P],
                                     df[:, 0, r0 * H:(r0 + 8) * H],
                                     start=True, stop=False)
                    nc.tensor.matmul(ops[:, mt * 512:(mt + 1) * 512],
                                     Pd_sb[:, i, 1, mt * P:(mt + 1) * P],
                                     df[:, 1, r0 * H:(r0 + 8) * H],
                                     start=False, stop=(s <= 2))
                    if s >= 3:
                        # accumulate the add-term via identity matmul (PE slack)
                        nc.tensor.matmul(ops[:, mt * 512:(mt + 1) * 512],
                                         IDb[:], add[:, mt],
                                         start=False, stop=True)
                src = ops[:].rearrange("p (m h w) -> p m h w", m=2, w=H)
                if final:
                    osb = opool.tile([P, 2, 8, 64], F32, tag="ob",
                                     name=f"ob_{b}_{blk}")
                    nc.scalar.copy(osb[:], src)
                    for mt in range(2):
                        nc.sync.dma_start(
                            out_ap[b, mt * P:(mt + 1) * P, r0:r0 + 8, :],
                            osb[:, mt])
                else:
                    if s == 1:      # row-type next: rows 0..63
                        dst = uN[:, :, r0:r0 + 8, 0:64]
                    elif s == 2:    # col-type shifted: cols 1..64
                        dst = uN[:, :, r0:r0 + 8, 1:65]
                    else:           # s == 3, row-type shifted: rows 1..64
                        dst = uN[:, :, r0 + 1:r0 + 9, 0:64]
                    if s <= 2:
                        nc.vector.tensor_add(dst, src, add)
                    else:
                        nc.scalar.copy(dst, src)
            if not final:
                for mt in range(2):
                    if s == 1:
                        nc.scalar.dma_start(uN[:, mt, 64:65, 0:64],
                                            uN[:, mt, 0:1, 0:64])
                    elif s == 2:
                        for hh in range(2):
                            r = slice(hh * 32, hh * 32 + 32)
                            nc.scalar.dma_start(uN[:, mt, r, 0:1],
                                                uN[:, mt, r, 64:65])
                    else:
                        nc.scalar.dma_start(uN[:, mt, 0:1, 0:64],
                                            uN[:, mt, 64:65, 0:64])
            return uN

        for pair in ((0, 1), (2, 3)):
            u0s, u1s, u2s, u3s, ds = {}, {}, {}, {}, {}
            for b in pair:
                u0s[b] = stage0(b)
                ds[b] = emit_diff(u0s[b], "col", b, 1)
            for b in pair:
                u1s[b] = stageP(u0s[b], ds[b], 0, "col", b, 1)
                ds[b] = emit_diff(u1s[b], "row", b, 2)
            for b in pair:
                u2s[b] = stageP(u1s[b], ds[b], 1, "row", b, 2)
                ds[b] = emit_diff(u2s[b], "col", b, 3)
            for b in pair:
                u3s[b] = stageP(u2s[b], ds[b], 2, "col", b, 3)
                ds[b] = emit_diff(u3s[b], "row", b, 4)
            for b in pair:
                stageP(u3s[b], ds[b], 3, "row", b, 4, final=True)


def build_program():
    from contextlib import ExitStack
    nc = bacc.Bacc("TRN2", target_bir_lowering=False, debug=False,
                   enable_asserts=False, num_devices=N_CORES)
    xs = nc.dram_tensor("xs", [B_CORE, C, H, H], F32R, kind="ExternalInput").ap()
    pmk = nc.dram_tensor("pmk", [C, C], F32R, kind="ExternalInput").ap()
    u0k = nc.dram_tensor("u0k", [C, 1], F32R, kind="ExternalInput").ap()
    bias = nc.dram_tensor("bias", [C], F32, kind="ExternalInput").ap()
    out = nc.dram_tensor("out", [B_CORE, C, H, H], F32, kind="ExternalOutput").ap()
    with tile.TileContext(nc) as tc:
        with ExitStack() as ctx:
            build_body(tc, out, xs, pmk, u0k, bias, ctx)
    nc.compile()
    return nc


_cached_nc = None


def make_in_maps(x, pm, u0, b):
    in_maps = []
    for i in range(N_CORES):
        k = i if i < NK else i - NK
        in_maps.append({
            "xs": np.ascontiguousarray(x[i * B_CORE:(i + 1) * B_CORE]),
            "pmk": np.ascontiguousarray(pm[k]),
            "u0k": np.ascontiguousarray(u0[k]),
            "bias": np.ascontiguousarray(b),
        })
    return in_maps


def kernel(x, param_matrices, init_u, bias):
    global _cached_nc
    if _cached_nc is None:
        _cached_nc = build_program()
    nc = _cached_nc
    x = np.ascontiguousarray(np.asarray(x, dtype=np.float32))
    pm = np.ascontiguousarray(np.asarray(param_matrices, dtype=np.float32))
    u0 = np.ascontiguousarray(np.asarray(init_u, dtype=np.float32))
    b = np.ascontiguousarray(np.asarray(bias, dtype=np.float32))
    in_maps = make_in_maps(x, pm, u0, b)
    res = run_bass_kernel_spmd(nc, in_maps, core_ids=list(range(N_CORES)))
    return np.concatenate([r["out"] for r in res.results], axis=0)


if __name__ == "__main__":
    import reference
    inputs = {k: np.asarray(v) for k, v in reference.setup_inputs().items()}
    out = kernel(**inputs)
    print(out.shape, out.dtype)
